# revision 17
# baseline (speedup 1.0000x reference)
"""Trainium2 Bass kernel for nn_CINLayer: out[b,d,o] = sum_{n,m} x[b,d,n]*y[b,d,m]*W[o,n*M+m].

Strategy (8-core data parallel over batch):
  Per sample s, out[o,s] = sum_k Wl[k,o] * Z[k,s] with Z[k,s] = x[s,n(k)]*y[s,m(k)].
  The contraction k (1600 products) is split into 13 chunks of 128 rows.
  The X-factor tile of each chunk (4 x-rows broadcast to 32 partitions each):
  chunks 2..12 are HOST-STAGED in replicated layout and DMA'd in; chunks 0..1
  are built on-chip by a DVE stream_shuffle from a tiny (8-row) staged tile —
  this balances DMA bytes (~10.4us/t2) against DVE time (~9.5us/t2) under the
  PE's 11.2us/t2.  Z chunks are one fp16 tensor_mul each (DVE; chunk 12 on
  GPSIMD), then feed fp16 matmuls accumulating out^T[o, s] in PSUM
  (o split 128+72, s tiles 512).  Chunks are processed in order 2..12,0,1 so
  the shuffle-built chunks are needed last within each t2 iteration.

  Chunk row mapping (r = 32j + r', j=quadrant):
    Part A (c<10):  (n, m) = (4c + j, r')          for r' < 32
    Part B (cb=c-10<3): r' = 8a + m''; (n, m) = (16cb + 4a + j, 32 + m'')
  Host layouts:
    xe[p, t2, ci, s'] = xT[n(ci+2, p), t2*1024+s']  (0 where padded)
    xq[j, i, t2, s'] = xT[4i + j, ...]  (i<2; lands on partition 32j+i)
    yab[p, t2, 0, s'] = yT[p % 32, ...]  (part A), [.,.,1,.] = yT[32 + p%8] (B)
  W rows with n >= 40 (part B overhang) are zeroed on host.
"""

import numpy as np

BS, DIM, N, M, O = 2048, 32, 40, 40, 200
NCORES = 8
S_PER_CORE = BS * DIM // NCORES  # 8192
S_TILE = 512
T2W = 2 * S_TILE  # 1024: samples per inner iteration
NT2 = S_PER_CORE // T2W  # 8
NCHUNKS = 13  # 10 part-A + 3 part-B
NSHUF = 2  # chunks 0..NSHUF-1 built by shuffle; the rest DMA'd
# processing order: folded chunk 12 last (its inputs arrive last)
PERM = list(range(NSHUF, NCHUNKS - 1)) + list(range(NSHUF)) + [12]
F16 = np.float16

# chunks whose Z-multiply runs on GPSIMD instead of DVE (DVE relief).
GPSIMD_MULS = frozenset({12})


def _chunk_row_to_nm(c: int, r: int):
    """Global chunk c (0..12), row r (0..127) -> (n, m) or None (zero pad)."""
    j, rp = divmod(r, 32)
    if c < 10:
        return 4 * c + j, rp
    cb = c - 10
    a, mpp = divmod(rp, 8)
    n = 16 * cb + 4 * a + j
    if n >= N:
        return None
    return n, 32 + mpp


def _n_index():
    idx = np.full((NCHUNKS, 128), -1, dtype=np.int64)
    for c in range(NCHUNKS):
        for r in range(128):
            nm = _chunk_row_to_nm(c, r)
            if nm is not None:
                idx[c, r] = nm[0]
    return idx


_N_IDX = _n_index()


def _stage_w(W: np.ndarray) -> np.ndarray:
    """W [O, N*M] f32 -> wl [128, NCHUNKS, O] f16 (lhsT layout per chunk).

    Chunk 12 (64 real rows) is stored folded: rows 64h+q (q=8b+m'') hold
    Wr[:, 32+b, 32+m''] for both sample-half row groups h."""
    Wr = W.reshape(O, N, M)
    wl = np.zeros((128, NCHUNKS, O), dtype=F16)
    for c in range(NCHUNKS - 1):
        for r in range(128):
            nm = _chunk_row_to_nm(c, r)
            if nm is not None:
                wl[r, c, :] = Wr[:, nm[0], nm[1]].astype(F16)
    q = np.arange(64)
    w12 = Wr[:, 32 + q // 8, 32 + q % 8].T.astype(F16)  # [64, O]
    wl[0:64, 12, :] = w12
    wl[64:128, 12, :] = w12
    return wl


def _stage_core_inputs(x_flat: np.ndarray, y_flat: np.ndarray):
    """[S_PER_CORE, 40] f32 x2 -> xe, xq, yab staged f16 tensors."""
    xT = np.ascontiguousarray(x_flat.T).astype(F16)  # [40, S]
    yT = np.ascontiguousarray(y_flat.T).astype(F16)  # [40, S]
    idx = _N_IDX[NSHUF : NCHUNKS - 1]  # DMA'd full chunks (folded c12 apart)
    xe = xT[np.clip(idx, 0, None)]
    xe[idx < 0] = 0
    xe = np.ascontiguousarray(
        xe.reshape(NCHUNKS - 1 - NSHUF, 128, NT2, T2W).transpose(1, 2, 0, 3)
    )  # [128, NT2, 10, T2W]
    # folded chunk 12: partition 64h+q (q=8b+m'') <- x[32+b]*y[32+m''] at
    # sample t2*1024 + 512h + s'
    q = np.arange(64)
    x12 = xT[32 + q // 8].reshape(64, NT2, 2, S_TILE)
    y12 = yT[32 + q % 8].reshape(64, NT2, 2, S_TILE)
    xe12 = np.ascontiguousarray(
        np.concatenate([x12[:, :, 0], x12[:, :, 1]], axis=0)
    )  # [128, NT2, 512]
    ye12 = np.ascontiguousarray(
        np.concatenate([y12[:, :, 0], y12[:, :, 1]], axis=0)
    )
    # xq[j, i] = xT[4i + j]: source rows for the shuffle-built chunks i<NSHUF
    xq = np.empty((4, NSHUF, NT2 * T2W), dtype=F16)
    for j in range(4):
        for i in range(NSHUF):
            xq[j, i] = xT[4 * i + j]
    ya = yT[np.arange(128) % 32]  # [128, S]
    yb = yT[32 + (np.arange(128) % 8)]
    yab = np.ascontiguousarray(
        np.stack([ya, yb], axis=1).reshape(128, 2, NT2, T2W).transpose(0, 2, 1, 3)
    )  # [128, NT2, 2, T2W]
    return xe, xq, yab, xe12, ye12


def _stage_boot(yab, xe, wl):
    """One-DMA boot bundle: [yab(t2=0) | xe(t2=0, chunk idx 0) | wl_c2]."""
    boot = np.concatenate(
        [yab[:, 0].reshape(128, 2 * T2W), xe[:, 0, 0], wl[:, NSHUF, :]], axis=1
    )
    return np.ascontiguousarray(boot)


def _stage_all(x: np.ndarray, y: np.ndarray, W: np.ndarray):
    wl = _stage_w(W)
    x_cores = x.reshape(NCORES, S_PER_CORE, N)
    y_cores = y.reshape(NCORES, S_PER_CORE, M)
    in_maps = []
    for i in range(NCORES):
        xe, xq, yab, xe12, ye12 = _stage_core_inputs(x_cores[i], y_cores[i])
        boot = _stage_boot(yab, xe, wl)
        in_maps.append({
            "xe": xe, "xq": xq, "yab": yab, "wl": wl, "boot": boot,
            "xe12": xe12, "ye12": ye12,
        })
    return in_maps


def build_nc(n_t2: int = NT2, debug: bool = False):
    """Build the per-core Bass/Tile module. Returns nc."""
    import concourse.bass as bass
    import concourse.tile as tile
    from concourse import bacc, mybir

    f16 = mybir.dt.float16
    f32 = mybir.dt.float32
    s_len = n_t2 * T2W
    nxe = NCHUNKS - 1 - NSHUF  # full chunks DMA'd (folded c12 separate)

    nc = bacc.Bacc("TRN2", target_bir_lowering=False, debug=debug)

    xe_d = nc.dram_tensor("xe", [128, n_t2, nxe, T2W], f16, kind="ExternalInput")
    xq_d = nc.dram_tensor("xq", [4, NSHUF, n_t2 * T2W], f16, kind="ExternalInput")
    boot_d = nc.dram_tensor("boot", [128, 3 * T2W + O], f16, kind="ExternalInput")
    xe12_d = nc.dram_tensor("xe12", [128, n_t2, S_TILE], f16, kind="ExternalInput")
    ye12_d = nc.dram_tensor("ye12", [128, n_t2, S_TILE], f16, kind="ExternalInput")
    yab_d = nc.dram_tensor("yab", [128, n_t2, 2, T2W], f16, kind="ExternalInput")
    wl_d = nc.dram_tensor("wl", [128, NCHUNKS, O], f16, kind="ExternalInput")
    out_d = nc.dram_tensor("outt", [O, s_len], f16, kind="ExternalOutput")

    with tile.TileContext(nc) as tc:
        with (
            tc.tile_pool(name="wpool", bufs=1) as wpool,
            tc.tile_pool(name="inp0", bufs=1) as inp0,
            tc.tile_pool(name="inp", bufs=4) as inp,
            tc.tile_pool(name="xqp", bufs=4) as xqp,
            tc.tile_pool(name="xsp", bufs=4) as xsp,
            tc.tile_pool(name="zp", bufs=8) as zp,
            tc.tile_pool(name="outp", bufs=4) as outp,
            tc.tile_pool(name="ps", bufs=2, space=bass.MemorySpace.PSUM) as psp,
        ):
            wl_sb = wpool.tile([128, NCHUNKS, O], f16)
            # xq is tiny: resident for the whole run, loaded once at boot.
            xq_all = wpool.tile([128, n_t2 * T2W], f16)
            boot = wpool.tile([128, 3 * T2W + O], f16)

            # dummy matmuls during the DMA lead-in keep the PE HAM activity
            # monitor busy so the real stream starts at 2.4GHz, not 1.2.
            scratch = wpool.tile([128, S_TILE], f16)
            nc.gpsimd.memset(scratch[:], 0)
            psW = psp.tile([128, S_TILE], f32, tag="psA0")
            for k in range(12):
                nc.tensor.matmul(
                    psW[:], scratch[:, 0:128], scratch[:],
                    start=k == 0, stop=k == 11,
                )

            for t2 in range(n_t2):
                if t2 == 0:
                    # one bundled DMA carries yab(t2=0) + xe chunk2 + wl_c2:
                    # the first 4 matmuls depend on just this one transfer.
                    nc.sync.dma_start(boot[:], boot_d[:])
                    yt = None  # t2=0 reads y from the boot bundle
                    xet = inp0.tile([128, nxe, T2W], f16, tag="xet0")
                    nc.sync.dma_start(xet[:, 1], xe_d[:, t2, 1])
                    # weights for the next few chunks first, then the rest
                    lo, hi = NSHUF + 1, NSHUF + 4
                    nc.sync.dma_start(wl_sb[:, lo:hi], wl_d[:, lo:hi])
                    nc.sync.dma_start(xet[:, 2], xe_d[:, t2, 2])
                    nc.sync.dma_start(wl_sb[:, hi:], wl_d[:, hi:])
                    nc.sync.dma_start(wl_sb[:, 0:lo], wl_d[:, 0:lo])
                    nc.sync.dma_start(xet[:, 3], xe_d[:, t2, 3])
                    for j in range(4):
                        nc.sync.dma_start(
                            xq_all[32 * j : 32 * j + NSHUF, :], xq_d[j]
                        )
                    for ci in range(4, nxe):
                        nc.sync.dma_start(xet[:, ci], xe_d[:, t2, ci])
                    x12t = inp.tile([128, S_TILE], f16, tag="x12")
                    nc.sync.dma_start(x12t[:], xe12_d[:, t2])
                    y12t = inp.tile([128, S_TILE], f16, tag="y12")
                    nc.sync.dma_start(y12t[:], ye12_d[:, t2])
                else:
                    # per-chunk DMAs everywhere: chunk-granular deps make the
                    # PE wait at most one chunk behind the DMA cursor and
                    # avoid whole-slab completion cliffs.
                    yt = inp.tile([128, 2, T2W], f16, tag="yt")
                    nc.sync.dma_start(yt[:], yab_d[:, t2])
                    xet = inp.tile([128, nxe, T2W], f16, tag="xet")
                    for ci in range(nxe):
                        nc.sync.dma_start(xet[:, ci], xe_d[:, t2, ci])
                    x12t = inp.tile([128, S_TILE], f16, tag="x12")
                    nc.sync.dma_start(x12t[:], xe12_d[:, t2])
                    y12t = inp.tile([128, S_TILE], f16, tag="y12")
                    nc.sync.dma_start(y12t[:], ye12_d[:, t2])

                psA0 = psp.tile([128, S_TILE], f32, tag="psA0")
                psB0 = psp.tile([72, S_TILE], f32, tag="psB0")
                psA1 = psp.tile([128, S_TILE], f32, tag="psA1")
                psB1 = psp.tile([72, S_TILE], f32, tag="psB1")
                ps = [psA0, psB0, psA1, psB1]
                for ic, c in enumerate(PERM):
                    if c == 12:
                        # folded: both sample halves in partition halves,
                        # K=64 row-tiled matmul pairs run concurrently.
                        z12 = zp.tile([128, S_TILE], f16, tag="z12")
                        nc.gpsimd.tensor_mul(z12[:], y12t[:], x12t[:])
                        for og in range(2):
                            osl = slice(0, 128) if og == 0 else slice(128, 200)
                            for h in range(2):
                                rs = slice(64 * h, 64 * h + 64)
                                nc.tensor.matmul(
                                    ps[2 * h + og][:], wl_sb[rs, 12, osl],
                                    z12[rs, :],
                                    start=False, stop=og == 1,
                                    tile_position=(64 * h, 0),
                                )
                        continue
                    if c < NSHUF:
                        xsl = xsp.tile([128, T2W], f16, tag="xs")
                        nc.vector.stream_shuffle(
                            xsl[:], xq_all[:, t2 * T2W : (t2 + 1) * T2W], [c] * 32
                        )
                        xsl = xsl[:]
                    elif t2 == 0 and ic == 0:
                        xsl = boot[:, 2 * T2W : 3 * T2W]
                    else:
                        xsl = xet[:, c - NSHUF, :]
                    z = zp.tile([128, T2W], f16)
                    if yt is None:
                        ysl = boot[:, 0:T2W] if c < 10 else boot[:, T2W : 2 * T2W]
                    else:
                        ysl = yt[:, 0 if c < 10 else 1, :]
                    eng = nc.gpsimd if c in GPSIMD_MULS else nc.vector
                    eng.tensor_mul(z[:], ysl, xsl)
                    first, last = ic == 0, ic == NCHUNKS - 1
                    if t2 == 0 and ic == 0:
                        wA = boot[:, 3 * T2W : 3 * T2W + 128]
                        wB = boot[:, 3 * T2W + 128 : 3 * T2W + O]
                    else:
                        wA = wl_sb[:, c, 0:128]
                        wB = wl_sb[:, c, 128:200]
                    for h in range(2):
                        zh = z[:, h * S_TILE : (h + 1) * S_TILE]
                        nc.tensor.matmul(
                            ps[2 * h][:], wA, zh, start=first, stop=last,
                        )
                        nc.tensor.matmul(
                            ps[2 * h + 1][:], wB, zh, start=first, stop=last,
                        )

                for h in range(2):
                    sl = bass.ts(2 * t2 + h, S_TILE)
                    # split the last iteration's copies across Scalar and
                    # Vector so the final PSUM drain is parallel (tail).
                    oA = outp.tile([128, S_TILE], f16, tag="oA")
                    nc.scalar.copy(oA[:], ps[2 * h][:])
                    oB = outp.tile([72, S_TILE], f16, tag="oB")
                    if t2 == n_t2 - 1:
                        nc.vector.tensor_copy(oB[:], ps[2 * h + 1][:])
                    else:
                        nc.scalar.copy(oB[:], ps[2 * h + 1][:])
                    if t2 == n_t2 - 1:
                        nc.sync.dma_start(out_d[0:128, sl], oA[:])
                    else:
                        nc.scalar.dma_start(out_d[0:128, sl], oA[:])
                    nc.scalar.dma_start(out_d[128:200, sl], oB[:])

    nc.compile()
    return nc


def kernel(x: np.ndarray, y: np.ndarray, W: np.ndarray) -> np.ndarray:
    from concourse.bass_utils import run_bass_kernel_spmd

    assert x.shape == (BS, DIM, N) and y.shape == (BS, DIM, M)
    assert W.shape == (O, N * M)

    in_maps = _stage_all(x, y, W)
    nc = build_nc()
    res = run_bass_kernel_spmd(nc, in_maps, core_ids=list(range(NCORES)))

    outs = []
    for i in range(NCORES):
        outt = res.results[i]["outt"]  # [O, S_PER_CORE] f16
        outs.append(outt.T.astype(np.float32))  # [S_PER_CORE, O]
    return np.concatenate(outs, axis=0).reshape(BS, DIM, O)


if __name__ == "__main__":
    xs = np.random.randn(BS, DIM, N).astype(np.float32)
    ys = np.random.randn(BS, DIM, M).astype(np.float32)
    Ws = (np.random.randn(O, N * M) * (1.0 / np.sqrt(N * M))).astype(np.float32)
    out = kernel(xs, ys, Ws)
    print(out.shape, out.dtype)


# revision 18
# speedup vs baseline: 1.0110x; 1.0110x over previous
"""Trainium2 Bass kernel for nn_CINLayer: out[b,d,o] = sum_{n,m} x[b,d,n]*y[b,d,m]*W[o,n*M+m].

Strategy (8-core data parallel over batch):
  Per sample s, out[o,s] = sum_k Wl[k,o] * Z[k,s] with Z[k,s] = x[s,n(k)]*y[s,m(k)].
  The contraction k (1600 products) is split into 13 chunks of 128 rows.
  The X-factor tile of each chunk (4 x-rows broadcast to 32 partitions each):
  chunks 2..12 are HOST-STAGED in replicated layout and DMA'd in; chunks 0..1
  are built on-chip by a DVE stream_shuffle from a tiny (8-row) staged tile —
  this balances DMA bytes (~10.4us/t2) against DVE time (~9.5us/t2) under the
  PE's 11.2us/t2.  Z chunks are one fp16 tensor_mul each (DVE; chunk 12 on
  GPSIMD), then feed fp16 matmuls accumulating out^T[o, s] in PSUM
  (o split 128+72, s tiles 512).  Chunks are processed in order 2..12,0,1 so
  the shuffle-built chunks are needed last within each t2 iteration.

  Chunk row mapping (r = 32j + r', j=quadrant):
    Part A (c<10):  (n, m) = (4c + j, r')          for r' < 32
    Part B (cb=c-10<3): r' = 8a + m''; (n, m) = (16cb + 4a + j, 32 + m'')
  Host layouts:
    xe[p, t2, ci, s'] = xT[n(ci+2, p), t2*1024+s']  (0 where padded)
    xq[j, i, t2, s'] = xT[4i + j, ...]  (i<2; lands on partition 32j+i)
    yab[p, t2, 0, s'] = yT[p % 32, ...]  (part A), [.,.,1,.] = yT[32 + p%8] (B)
  W rows with n >= 40 (part B overhang) are zeroed on host.
"""

import numpy as np

BS, DIM, N, M, O = 2048, 32, 40, 40, 200
NCORES = 8
S_PER_CORE = BS * DIM // NCORES  # 8192
S_TILE = 512
T2W = 2 * S_TILE  # 1024: samples per inner iteration
NT2 = S_PER_CORE // T2W  # 8
NCHUNKS = 13  # 10 part-A + 3 part-B
NSHUF = 2  # chunks 0..NSHUF-1 built by shuffle; the rest DMA'd
# processing order: folded chunk 12 last (its inputs arrive last)
PERM = list(range(NSHUF, NCHUNKS - 1)) + list(range(NSHUF)) + [12]
F16 = np.float16

# chunks whose Z-multiply runs on GPSIMD instead of DVE (DVE relief).
GPSIMD_MULS = frozenset({11})


def _chunk_row_to_nm(c: int, r: int):
    """Global chunk c (0..12), row r (0..127) -> (n, m) or None (zero pad)."""
    j, rp = divmod(r, 32)
    if c < 10:
        return 4 * c + j, rp
    cb = c - 10
    a, mpp = divmod(rp, 8)
    n = 16 * cb + 4 * a + j
    if n >= N:
        return None
    return n, 32 + mpp


def _n_index():
    idx = np.full((NCHUNKS, 128), -1, dtype=np.int64)
    for c in range(NCHUNKS):
        for r in range(128):
            nm = _chunk_row_to_nm(c, r)
            if nm is not None:
                idx[c, r] = nm[0]
    return idx


_N_IDX = _n_index()


def _stage_w(W: np.ndarray) -> np.ndarray:
    """W [O, N*M] f32 -> wl [128, NCHUNKS, O] f16 (lhsT layout per chunk).

    Chunk 12 (64 real rows) is stored folded: rows 64h+q (q=8b+m'') hold
    Wr[:, 32+b, 32+m''] for both sample-half row groups h."""
    Wr = W.reshape(O, N, M)
    wl = np.zeros((128, NCHUNKS, O), dtype=F16)
    for c in range(NCHUNKS - 1):
        for r in range(128):
            nm = _chunk_row_to_nm(c, r)
            if nm is not None:
                wl[r, c, :] = Wr[:, nm[0], nm[1]].astype(F16)
    q = np.arange(64)
    w12 = Wr[:, 32 + q // 8, 32 + q % 8].T.astype(F16)  # [64, O]
    wl[0:64, 12, :] = w12
    wl[64:128, 12, :] = w12
    return wl


def _stage_core_inputs(x_flat: np.ndarray, y_flat: np.ndarray):
    """[S_PER_CORE, 40] f32 x2 -> xe, xq, yab staged f16 tensors."""
    xT = np.ascontiguousarray(x_flat.T).astype(F16)  # [40, S]
    yT = np.ascontiguousarray(y_flat.T).astype(F16)  # [40, S]
    idx = _N_IDX[NSHUF : NCHUNKS - 1]  # DMA'd full chunks (folded c12 apart)
    xe = xT[np.clip(idx, 0, None)]
    xe[idx < 0] = 0
    xe = np.ascontiguousarray(
        xe.reshape(NCHUNKS - 1 - NSHUF, 128, NT2, T2W).transpose(1, 2, 0, 3)
    )  # [128, NT2, 10, T2W]
    # folded chunk 12: partition 64h+q (q=8b+m'') <- x[32+b]*y[32+m''] at
    # sample t2*1024 + 512h + s'
    q = np.arange(64)
    x12 = xT[32 + q // 8].reshape(64, NT2, 2, S_TILE)
    y12 = yT[32 + q % 8].reshape(64, NT2, 2, S_TILE)
    xe12 = np.ascontiguousarray(
        np.concatenate([x12[:, :, 0], x12[:, :, 1]], axis=0)
    )  # [128, NT2, 512]
    ye12 = np.ascontiguousarray(
        np.concatenate([y12[:, :, 0], y12[:, :, 1]], axis=0)
    )
    # xq[j, i] = xT[4i + j]: source rows for the shuffle-built chunks i<NSHUF
    xq = np.empty((4, NSHUF, NT2 * T2W), dtype=F16)
    for j in range(4):
        for i in range(NSHUF):
            xq[j, i] = xT[4 * i + j]
    ya = yT[np.arange(128) % 32]  # [128, S]
    yb = yT[32 + (np.arange(128) % 8)]
    yab = np.ascontiguousarray(
        np.stack([ya, yb], axis=1).reshape(128, 2, NT2, T2W).transpose(0, 2, 1, 3)
    )  # [128, NT2, 2, T2W]
    return xe, xq, yab, xe12, ye12


def _stage_boot(yab, xe, wl):
    """One-DMA boot bundle: [yab(t2=0) | xe(t2=0, chunk idx 0) | wl_c2]."""
    boot = np.concatenate(
        [yab[:, 0].reshape(128, 2 * T2W), xe[:, 0, 0], wl[:, NSHUF, :]], axis=1
    )
    return np.ascontiguousarray(boot)


def _stage_all(x: np.ndarray, y: np.ndarray, W: np.ndarray):
    wl = _stage_w(W)
    x_cores = x.reshape(NCORES, S_PER_CORE, N)
    y_cores = y.reshape(NCORES, S_PER_CORE, M)
    in_maps = []
    for i in range(NCORES):
        xe, xq, yab, xe12, ye12 = _stage_core_inputs(x_cores[i], y_cores[i])
        boot = _stage_boot(yab, xe, wl)
        in_maps.append({
            "xe": xe, "xq": xq, "yab": yab, "wl": wl, "boot": boot,
            "xe12": xe12, "ye12": ye12,
        })
    return in_maps


def build_nc(n_t2: int = NT2, debug: bool = False):
    """Build the per-core Bass/Tile module. Returns nc."""
    import concourse.bass as bass
    import concourse.tile as tile
    from concourse import bacc, mybir

    f16 = mybir.dt.float16
    f32 = mybir.dt.float32
    s_len = n_t2 * T2W
    nxe = NCHUNKS - 1 - NSHUF  # full chunks DMA'd (folded c12 separate)

    nc = bacc.Bacc("TRN2", target_bir_lowering=False, debug=debug)

    xe_d = nc.dram_tensor("xe", [128, n_t2, nxe, T2W], f16, kind="ExternalInput")
    xq_d = nc.dram_tensor("xq", [4, NSHUF, n_t2 * T2W], f16, kind="ExternalInput")
    boot_d = nc.dram_tensor("boot", [128, 3 * T2W + O], f16, kind="ExternalInput")
    xe12_d = nc.dram_tensor("xe12", [128, n_t2, S_TILE], f16, kind="ExternalInput")
    ye12_d = nc.dram_tensor("ye12", [128, n_t2, S_TILE], f16, kind="ExternalInput")
    yab_d = nc.dram_tensor("yab", [128, n_t2, 2, T2W], f16, kind="ExternalInput")
    wl_d = nc.dram_tensor("wl", [128, NCHUNKS, O], f16, kind="ExternalInput")
    out_d = nc.dram_tensor("outt", [O, s_len], f16, kind="ExternalOutput")

    with tile.TileContext(nc) as tc:
        with (
            tc.tile_pool(name="wpool", bufs=1) as wpool,
            tc.tile_pool(name="inp0", bufs=1) as inp0,
            tc.tile_pool(name="inp", bufs=4) as inp,
            tc.tile_pool(name="xqp", bufs=4) as xqp,
            tc.tile_pool(name="xsp", bufs=4) as xsp,
            tc.tile_pool(name="zp", bufs=8) as zp,
            tc.tile_pool(name="outp", bufs=4) as outp,
            tc.tile_pool(name="ps", bufs=2, space=bass.MemorySpace.PSUM) as psp,
        ):
            wl_sb = wpool.tile([128, NCHUNKS, O], f16)
            # xq is tiny: resident for the whole run, loaded once at boot.
            xq_all = wpool.tile([128, n_t2 * T2W], f16)
            boot = wpool.tile([128, 3 * T2W + O], f16)

            # dummy matmuls during the DMA lead-in keep the PE HAM activity
            # monitor busy so the real stream starts at 2.4GHz, not 1.2.
            scratch = wpool.tile([128, S_TILE], f16)
            nc.gpsimd.memset(scratch[:], 0)
            psW = psp.tile([128, S_TILE], f32, tag="psA0")
            for k in range(12):
                nc.tensor.matmul(
                    psW[:], scratch[:, 0:128], scratch[:],
                    start=k == 0, stop=k == 11,
                )

            for t2 in range(n_t2):
                if t2 == 0:
                    # one bundled DMA carries yab(t2=0) + xe chunk2 + wl_c2:
                    # the first 4 matmuls depend on just this one transfer.
                    nc.sync.dma_start(boot[:], boot_d[:])
                    yt = None  # t2=0 reads y from the boot bundle
                    xet = inp0.tile([128, nxe, T2W], f16, tag="xet0")
                    nc.sync.dma_start(xet[:, 1], xe_d[:, t2, 1])
                    # weights for the next few chunks first, then the rest
                    lo, hi = NSHUF + 1, NSHUF + 4
                    nc.sync.dma_start(wl_sb[:, lo:hi], wl_d[:, lo:hi])
                    nc.sync.dma_start(xet[:, 2], xe_d[:, t2, 2])
                    nc.sync.dma_start(wl_sb[:, hi:], wl_d[:, hi:])
                    nc.sync.dma_start(wl_sb[:, 0:lo], wl_d[:, 0:lo])
                    nc.sync.dma_start(xet[:, 3], xe_d[:, t2, 3])
                    for j in range(4):
                        nc.sync.dma_start(
                            xq_all[32 * j : 32 * j + NSHUF, :], xq_d[j]
                        )
                    for ci in range(4, nxe):
                        nc.sync.dma_start(xet[:, ci], xe_d[:, t2, ci])
                    x12t = inp.tile([128, S_TILE], f16, tag="x12")
                    nc.sync.dma_start(x12t[:], xe12_d[:, t2])
                    y12t = inp.tile([128, S_TILE], f16, tag="y12")
                    nc.sync.dma_start(y12t[:], ye12_d[:, t2])
                else:
                    # per-chunk DMAs everywhere: chunk-granular deps make the
                    # PE wait at most one chunk behind the DMA cursor and
                    # avoid whole-slab completion cliffs.
                    yt = inp.tile([128, 2, T2W], f16, tag="yt")
                    nc.sync.dma_start(yt[:], yab_d[:, t2])
                    xet = inp.tile([128, nxe, T2W], f16, tag="xet")
                    for ci in range(nxe):
                        nc.sync.dma_start(xet[:, ci], xe_d[:, t2, ci])
                    x12t = inp.tile([128, S_TILE], f16, tag="x12")
                    nc.sync.dma_start(x12t[:], xe12_d[:, t2])
                    y12t = inp.tile([128, S_TILE], f16, tag="y12")
                    nc.sync.dma_start(y12t[:], ye12_d[:, t2])

                psA0 = psp.tile([128, S_TILE], f32, tag="psA0")
                psB0 = psp.tile([72, S_TILE], f32, tag="psB0")
                psA1 = psp.tile([128, S_TILE], f32, tag="psA1")
                psB1 = psp.tile([72, S_TILE], f32, tag="psB1")
                ps = [psA0, psB0, psA1, psB1]
                for ic, c in enumerate(PERM):
                    if c == 12:
                        # folded: both sample halves in partition halves,
                        # K=64 row-tiled matmul pairs run concurrently.
                        z12 = zp.tile([128, S_TILE], f16, tag="z12")
                        nc.vector.tensor_mul(z12[:], y12t[:], x12t[:])
                        for og in range(2):
                            osl = slice(0, 128) if og == 0 else slice(128, 200)
                            for h in range(2):
                                rs = slice(64 * h, 64 * h + 64)
                                nc.tensor.matmul(
                                    ps[2 * h + og][:], wl_sb[rs, 12, osl],
                                    z12[rs, :],
                                    start=False, stop=True,
                                    tile_position=(64 * h, 0),
                                )
                        continue
                    if c < NSHUF:
                        xsl = xsp.tile([128, T2W], f16, tag="xs")
                        nc.vector.stream_shuffle(
                            xsl[:], xq_all[:, t2 * T2W : (t2 + 1) * T2W], [c] * 32
                        )
                        xsl = xsl[:]
                    elif t2 == 0 and ic == 0:
                        xsl = boot[:, 2 * T2W : 3 * T2W]
                    else:
                        xsl = xet[:, c - NSHUF, :]
                    z = zp.tile([128, T2W], f16)
                    if yt is None:
                        ysl = boot[:, 0:T2W] if c < 10 else boot[:, T2W : 2 * T2W]
                    else:
                        ysl = yt[:, 0 if c < 10 else 1, :]
                    eng = nc.gpsimd if c in GPSIMD_MULS else nc.vector
                    eng.tensor_mul(z[:], ysl, xsl)
                    first, last = ic == 0, ic == NCHUNKS - 1
                    if t2 == 0 and ic == 0:
                        wA = boot[:, 3 * T2W : 3 * T2W + 128]
                        wB = boot[:, 3 * T2W + 128 : 3 * T2W + O]
                    else:
                        wA = wl_sb[:, c, 0:128]
                        wB = wl_sb[:, c, 128:200]
                    for h in range(2):
                        zh = z[:, h * S_TILE : (h + 1) * S_TILE]
                        nc.tensor.matmul(
                            ps[2 * h][:], wA, zh, start=first, stop=last,
                        )
                        nc.tensor.matmul(
                            ps[2 * h + 1][:], wB, zh, start=first, stop=last,
                        )

                for h in range(2):
                    sl = bass.ts(2 * t2 + h, S_TILE)
                    # split the last iteration's copies across Scalar and
                    # Vector so the final PSUM drain is parallel (tail).
                    oA = outp.tile([128, S_TILE], f16, tag="oA")
                    nc.scalar.copy(oA[:], ps[2 * h][:])
                    oB = outp.tile([72, S_TILE], f16, tag="oB")
                    if t2 == n_t2 - 1:
                        nc.vector.tensor_copy(oB[:], ps[2 * h + 1][:])
                    else:
                        nc.scalar.copy(oB[:], ps[2 * h + 1][:])
                    if t2 == n_t2 - 1:
                        nc.sync.dma_start(out_d[0:128, sl], oA[:])
                    else:
                        nc.scalar.dma_start(out_d[0:128, sl], oA[:])
                    nc.scalar.dma_start(out_d[128:200, sl], oB[:])

    nc.compile()
    return nc


def kernel(x: np.ndarray, y: np.ndarray, W: np.ndarray) -> np.ndarray:
    from concourse.bass_utils import run_bass_kernel_spmd

    assert x.shape == (BS, DIM, N) and y.shape == (BS, DIM, M)
    assert W.shape == (O, N * M)

    in_maps = _stage_all(x, y, W)
    nc = build_nc()
    res = run_bass_kernel_spmd(nc, in_maps, core_ids=list(range(NCORES)))

    outs = []
    for i in range(NCORES):
        outt = res.results[i]["outt"]  # [O, S_PER_CORE] f16
        outs.append(outt.T.astype(np.float32))  # [S_PER_CORE, O]
    return np.concatenate(outs, axis=0).reshape(BS, DIM, O)


if __name__ == "__main__":
    xs = np.random.randn(BS, DIM, N).astype(np.float32)
    ys = np.random.randn(BS, DIM, M).astype(np.float32)
    Ws = (np.random.randn(O, N * M) * (1.0 / np.sqrt(N * M))).astype(np.float32)
    out = kernel(xs, ys, Ws)
    print(out.shape, out.dtype)
